# revision 3
# baseline (speedup 1.0000x reference)
"""Trainium2 Bass kernel: Whisper-style self-attention (B=4, S=1500, D=1280, H=20).

Sharding: core c = 2*b + g handles batch b (of 4) and head-group g (of 2,
10 heads each).  Every matmul is exactly 1/8 of the total work:
  - Q/K/V projections column-sharded over the head group,
  - attention sharded by (batch, head),
  - output projection row-sharded; the two head-group partials of each batch
    are summed on the host (plus bias terms, which fold into host math).

Device dataflow (per core), all fp16 operands (PSUM f32):
  xT [1280,1500] -> qT,kT [640,1500] fp16 (qT scaled 1/8 + bq),
  v [1500,10,65] (64 v cols + ones col per head -> softmax Z).
  Per (head h, sq chunk c): scoresT = kT.T@qT per (128-row k tile), Exp
  batched over psum bank pairs on ACT -> expT fp16.  Then per 128-col sq
  subtile: ctx[sq,65] accumulated in PSUM with ex as the STATIONARY operand
  (12 matmuls of only 65 moving cols each - 2x fewer PE cycles than
  streaming expT), DVE reciprocal of the Z column + per-partition
  tensor_scalar multiply -> ctx_sb fp16 [sq,128] (head pair), then a DMA
  transpose (xbar, 14ns/tile) writes ctxT [128,sq] directly - no PE
  transpose, no PSUM->SBUF copy.
  O-proj fp16 (ctxT stationary, wo moving) -> out [1500,1280] fp16.
Emission interleaves projections/O-proj between attention units so ACT
(exp, the attention-phase bottleneck) never starves PE.
"""
import sys
sys.path.insert(0, "/opt/trn_rl_repo")

from contextlib import ExitStack
import numpy as np

import concourse.bass as bass
import concourse.tile as tile
from concourse import bacc, mybir
from concourse.bass_utils import run_bass_kernel_spmd

dt = mybir.dt
AF = mybir.ActivationFunctionType
ALU = mybir.AluOpType

N_CORES = 8
B, S, D = 4, 1500, 1280
H, DH = 20, 64
G = 2
DG = D // G           # 640
HPG = H // G          # 10
KD = D // 128         # 10
MD = DG // 128        # 5
CW = (512, 512, 476)  # sq/proj chunk widths (PSUM-bank bound)
CO = (0, 512, 1024)   # chunk offsets
NS = 3
KS = (S + 127) // 128  # 12 (11*128 + 92)
ON = (512, 512, 256)
SP = S + 4            # ctxT padded to 1504 so the last 96-wide sq subtile
                      # (92 real cols) can be DMA-transposed whole

_CACHE = {}


def _sk(i):
    return min(128, S - i * 128)


def _subtiles(c):
    """(local_off, width) 128-col subtiles of chunk c; last one padded to 96."""
    w = CW[c]
    out = []
    off = 0
    while off < w:
        sw = min(128, w - off)
        if sw % 16:
            sw = 96  # pad 92 -> 96 for the xbar transpose (junk cols unread)
        out.append((off, sw))
        off += 128
    return out


def build():
    nc = bacc.Bacc("TRN2", target_bir_lowering=False, debug=False,
                   num_devices=N_CORES)
    xt_d = nc.dram_tensor("xt", [D, S], dt.float16, kind="ExternalInput").ap()
    wq_d = nc.dram_tensor("wq", [D, DG], dt.float16, kind="ExternalInput").ap()
    wk_d = nc.dram_tensor("wk", [D, DG], dt.float16, kind="ExternalInput").ap()
    wv_d = nc.dram_tensor("wv", [D, DG], dt.float16, kind="ExternalInput").ap()
    wo_d = nc.dram_tensor("wo", [DG, D], dt.float16, kind="ExternalInput").ap()
    bq_d = nc.dram_tensor("bq", [128, MD], dt.float32, kind="ExternalInput").ap()
    out_d = nc.dram_tensor("out", [S, D], dt.float16, kind="ExternalOutput").ap()

    xt_r = xt_d.rearrange("(k p) s -> p k s", p=128)
    wq_r = wq_d.rearrange("(k p) n -> p k n", p=128)
    wk_r = wk_d.rearrange("(k p) n -> p k n", p=128)
    wv_r = wv_d.rearrange("(k p) n -> p k n", p=128)
    wo_r = wo_d.rearrange("(k p) n -> p k n", p=128)

    with tile.TileContext(nc) as tc, ExitStack() as octx:
        persist = octx.enter_context(tc.tile_pool(name="persist", bufs=1))
        epool = octx.enter_context(tc.tile_pool(name="expT", bufs=3))
        zpool = octx.enter_context(tc.tile_pool(name="z", bufs=3))
        cpool = octx.enter_context(tc.tile_pool(name="ctxsb", bufs=6))
        ps2 = octx.enter_context(tc.tile_pool(name="ps2", bufs=2, space="PSUM"))
        ps1 = octx.enter_context(tc.tile_pool(name="ps1", bufs=2, space="PSUM"))
        pat = octx.enter_context(tc.tile_pool(name="pat", bufs=2, space="PSUM"))

        qT = persist.tile([128, MD, S], dt.float16, tag="qT")
        kT = persist.tile([128, MD, S], dt.float16, tag="kT")
        v = persist.tile([128, KS, HPG, DH + 1], dt.float16, tag="v")
        ctxT = persist.tile([128, MD, SP], dt.float16, tag="ctxT")
        bq_s = persist.tile([128, MD], dt.float32, tag="bq")

        nc.sync.dma_start(out=bq_s[:], in_=bq_d[:])
        ones1 = persist.tile([128, 1], dt.float16, tag="ones1")
        nc.vector.memset(ones1[:], 1.0)
        nc.vector.tensor_copy(v[:, :, :, DH:DH + 1],
                              ones1[:].to_broadcast([128, KS, HPG, 1]))

        pb = ExitStack()
        xpool = pb.enter_context(tc.tile_pool(name="xt", bufs=1))
        wst = pb.enter_context(tc.tile_pool(name="wst", bufs=2))

        xt_s = xpool.tile([128, KD, S], dt.float16, tag="xt")
        for n in range(NS):
            nsl = slice(CO[n], CO[n] + CW[n])
            nc.sync.dma_start(out=xt_s[:, :, nsl], in_=xt_r[:, :, nsl])

        def emit_qk(m):
            """qT and kT for d-tile m (heads 2m, 2m+1)."""
            for w_r, dst, is_q in ((wq_r, qT, True), (wk_r, kT, False)):
                wt = wst.tile([128, KD, 320], dt.float16, tag="wst")
                nc.scalar.dma_start(out=wt[:, :, 0:128],
                                    in_=w_r[:, :, m * 128:(m + 1) * 128])
                for n in range(NS):
                    cw, co = CW[n], CO[n]
                    ps = ps1.tile([128, 1, 512], dt.float32, tag="ps1")
                    for kk in range(KD):
                        nc.tensor.matmul(
                            ps[:, 0, 0:cw],
                            lhsT=wt[:, kk, 0:128],
                            rhs=xt_s[:, kk, co:co + cw],
                            start=(kk == 0), stop=(kk == KD - 1))
                    osl = dst[:, m, co:co + cw]
                    if is_q:
                        nc.vector.tensor_scalar(
                            osl, ps[:, 0, 0:cw], 0.125, bq_s[:, m:m + 1],
                            op0=ALU.mult, op1=ALU.add)
                    else:
                        nc.vector.tensor_copy(osl, ps[:, 0, 0:cw])

        def emit_v(n):
            """v columns for heads 5n..5n+4 (+ their ones cols untouched)."""
            wt = wst.tile([128, KD, 320], dt.float16, tag="wst")
            nc.scalar.dma_start(out=wt[:], in_=wv_r[:, :, n * 320:(n + 1) * 320])
            for ms in range(KS):
                sp = _sk(ms)
                ps = ps1.tile([128, 1, 512], dt.float32, tag="ps1")
                for kk in range(KD):
                    nc.tensor.matmul(
                        ps[0:sp, 0, 0:320],
                        lhsT=xt_s[:, kk, ms * 128:ms * 128 + sp],
                        rhs=wt[:, kk, :],
                        start=(kk == 0), stop=(kk == KD - 1))
                nc.vector.tensor_copy(
                    v[0:sp, ms, n * 5:(n + 1) * 5, 0:DH],
                    ps[0:sp, 0, 0:320].rearrange("p (h e) -> p h e", h=5))

        def emit_scores(h, c):
            """scoresT + exp for head h, sq chunk c -> ex tile [k, 12, q]."""
            base = 64 * (h % 2)
            td = h // 2
            cw, co = CW[c], CO[c]
            csl = slice(co, co + cw)
            ex = epool.tile([128, KS, 512], dt.float16, tag="expT")
            for kk2 in range(0, KS, 2):
                ps = ps2.tile([128, 2, 512], dt.float32, tag="ps2")
                for j in range(2):
                    kk = kk2 + j
                    sp = _sk(kk)
                    nc.tensor.matmul(
                        ps[0:sp, j, 0:cw],
                        lhsT=kT[base:base + 64, td, kk * 128:kk * 128 + sp],
                        rhs=qT[base:base + 64, td, csl],
                        start=True, stop=True)
                nc.scalar.activation(ex[:, kk2:kk2 + 2, 0:cw], ps[:, :, 0:cw],
                                     AF.Exp)
            return ex

        def emit_tail(h, c, ex, csb):
            """ctx for head h chunk c: attnV (ex stationary), 1/Z scale into
            ctx_sb col half; csb maps local subtile offset -> ctx_sb tile."""
            hb = 64 * (h % 2)
            for off, sw in _subtiles(c):
                pc = pat.tile([128, DH + 1], dt.float32, tag="pat")
                for kk in range(KS):
                    sp = _sk(kk)
                    nc.tensor.matmul(
                        pc[0:sw, :],
                        lhsT=ex[0:sp, kk, off:off + sw],
                        rhs=v[0:sp, kk, h, :],
                        start=(kk == 0), stop=(kk == KS - 1))
                rz = zpool.tile([128, 1], dt.float32, tag="rz")
                nc.vector.reciprocal(rz[0:sw, :], pc[0:sw, DH:DH + 1])
                nc.vector.tensor_scalar(
                    csb[off][0:sw, hb:hb + 64], pc[0:sw, 0:DH], rz[0:sw, :],
                    None, op0=ALU.mult)

        def emit_unitpair(td, c):
            """Both heads of pair td over chunk c, then DMA-transpose ctx."""
            csb = {off: cpool.tile([128, 128], dt.float16, tag="ctxsb",
                                   name="ctxsb")
                   for off, _ in _subtiles(c)}
            for h in (2 * td, 2 * td + 1):
                ex = emit_scores(h, c)
                emit_tail(h, c, ex, csb)
            for off, sw in _subtiles(c):
                nc.sync.dma_start(
                    out=ctxT[:, td, CO[c] + off:CO[c] + off + sw],
                    in_=csb[off][0:sw, :], transpose=True)

        emitted_oproj = [False] * KS

        def emit_oproj(ms_range, wo_s, opool):
            for ms in ms_range:
                if emitted_oproj[ms]:
                    continue
                emitted_oproj[ms] = True
                sp = _sk(ms)
                noff = 0
                for nw in ON:
                    ps = ps1.tile([128, 1, 512], dt.float32, tag="ps1")
                    for kk in range(MD):
                        nc.tensor.matmul(
                            ps[0:sp, 0, 0:nw],
                            lhsT=ctxT[:, kk, ms * 128:ms * 128 + sp],
                            rhs=wo_s[:, kk, noff:noff + nw],
                            start=(kk == 0), stop=(kk == MD - 1))
                    ob = opool.tile([128, 512], dt.float16, tag="ob")
                    nc.vector.tensor_copy(ob[0:sp, 0:nw], ps[0:sp, 0, 0:nw])
                    nc.sync.dma_start(
                        out=out_d[ms * 128:ms * 128 + sp, noff:noff + nw],
                        in_=ob[0:sp, 0:nw])
                    noff += nw

        # ---- interleaved emission: projections ride along with attention
        # units so PE has filler while ACT burns through the exps.
        emit_qk(0)
        emit_qk(1)
        emit_v(0)
        emit_unitpair(0, 0)
        emit_qk(2)
        emit_unitpair(0, 1)
        emit_v(1)
        emit_unitpair(1, 0)
        emit_qk(3)
        emit_unitpair(1, 1)
        emit_qk(4)
        emit_unitpair(2, 0)
        emit_unitpair(2, 1)
        emit_unitpair(3, 0)
        emit_unitpair(3, 1)
        emit_unitpair(4, 0)
        pb.close()  # free xt + weight streaming space

        pdx = ExitStack()
        wopool = pdx.enter_context(tc.tile_pool(name="wo", bufs=1))
        opool = pdx.enter_context(tc.tile_pool(name="ob", bufs=3))
        wo_s = wopool.tile([128, MD, D], dt.float16, tag="wo")
        nc.gpsimd.dma_start(out=wo_s[:], in_=wo_r[:])

        emit_unitpair(4, 1)
        emit_oproj(range(0, 4), wo_s, opool)     # sq < 512 final after c=0
        emit_unitpair(0, 2)
        emit_oproj(range(4, 8), wo_s, opool)     # sq < 1024 final after c=1
        emit_unitpair(1, 2)
        emit_unitpair(2, 2)
        emit_unitpair(3, 2)
        emit_unitpair(4, 2)
        emit_oproj(range(8, KS), wo_s, opool)
        pdx.close()

    nc.compile()
    return nc


def _get_nc():
    if "nc" not in _CACHE:
        _CACHE["nc"] = build()
    return _CACHE["nc"]


def _prep_in_maps(x, Wq, bq, Wk, Wv, Wo):
    in_maps = []
    for c in range(N_CORES):
        b, g = divmod(c, G)
        gs = slice(g * DG, (g + 1) * DG)
        in_maps.append({
            "xt": np.ascontiguousarray(x[b].T).astype(np.float16),
            "wq": np.ascontiguousarray(Wq[gs, :].T).astype(np.float16),
            "wk": np.ascontiguousarray(Wk[gs, :].T).astype(np.float16),
            "wv": np.ascontiguousarray(Wv[gs, :].T).astype(np.float16),
            "wo": np.ascontiguousarray(Wo[:, gs].T).astype(np.float16),
            "bq": np.ascontiguousarray(
                (0.125 * bq[gs]).astype(np.float32).reshape(MD, 128).T),
        })
    return in_maps


def run(x, Wq, bq, Wk, Wv, bv, Wo, bo, trace=False, **trace_kw):
    x = np.asarray(x, dtype=np.float32)
    Wq = np.asarray(Wq, dtype=np.float32)
    bq = np.asarray(bq, dtype=np.float32)
    Wk = np.asarray(Wk, dtype=np.float32)
    Wv = np.asarray(Wv, dtype=np.float32)
    bv = np.asarray(bv, dtype=np.float32)
    Wo = np.asarray(Wo, dtype=np.float32)
    bo = np.asarray(bo, dtype=np.float32)

    nc = _get_nc()
    in_maps = _prep_in_maps(x, Wq, bq, Wk, Wv, Wo)
    res = None
    for attempt in range(3):
        try:
            res = run_bass_kernel_spmd(nc, in_maps, list(range(N_CORES)),
                                       trace=trace, **trace_kw)
            break
        except Exception:
            # Sporadic NRT_EXEC_UNIT_UNRECOVERABLE on first exec; devices
            # come back after ~75s. Reset the backend and retry.
            if attempt == 2:
                raise
            import time as _time
            import jax as _jax
            _time.sleep(80)
            try:
                _jax.clear_backends()
            except Exception:
                pass
    const = (bv @ Wo.T + bo).astype(np.float32)  # [D]
    out = np.empty((B, S, D), dtype=np.float32)
    for b in range(B):
        out[b] = (res.results[2 * b]["out"].astype(np.float32)
                  + res.results[2 * b + 1]["out"].astype(np.float32) + const)
    return out, res


def kernel(**inputs):
    out, _ = run(**inputs)
    return out
